# revision 7
# baseline (speedup 1.0000x reference)
"""Trainium2 Bass kernel for nn_AlignmentLoss (triplet + CE over phrase/input embeddings).

Sharding: batch dimension N=128 split 16 batches/core across 8 cores.  Each core
owns the positive pairs whose batch_idxs falls in its range (host buckets pairs,
padded to a fixed per-batch capacity cap=64; 2 batches share a 128-partition tile).

v6 design:
 - Host L2-normalizes phrase and input embeddings in f32 (exactly the
   reference's F.normalize preprocessing), so the device never computes
   norms: no squares, no ones-matmuls, no rsqrts, no row rescaling.
 - All matmul + dot operands ship as fp8 (e4m3): halves the big HBM
   transfer; cos rows come straight out of the PE into PSUM and are
   consumed by DVE Max8 directly from PSUM (no copies).
 - Triplet dot products (anchor*pos, anchor*rng) run as per-tile GpSimd
   elementwise multiplies (hidden under the sim pipeline) + per-tile DVE
   grouped reduces interleaved between the Max8s.
 - CE: fp8 matmul logits -> ACT Exp(scale=T) with accum_out.  The
   log-softmax finale (ln, valid-masking, means) runs on the host from
   per-pair statistics [trip | sumexp] + sdots shipped back (tiny DMAs),
   removing the Ln table load and the cross-partition reduction.
 - DMA issues spread across Sync/Vector/Pool sequencers, small CE/sim
   stationaries first so the PE starts while xt still streams.
"""

import sys

for _p in ("/opt/trn_rl_repo", "/root/.axon_site/_ro/trn_rl_repo"):
    if _p not in sys.path:
        sys.path.append(_p)

import numpy as np

import concourse.bass as bass
import concourse.bacc as bacc
import concourse.mybir as mybir
from concourse.tile import TileContext
from concourse.bass_utils import run_bass_kernel_spmd

F32 = mybir.dt.float32
BF16 = mybir.dt.bfloat16
FP8 = mybir.dt.float8e4
AF = mybir.ActivationFunctionType
ALU = mybir.AluOpType
AX = mybir.AxisListType

N, K, M, D, P = 128, 1024, 512, 128, 4096
NCORES = 8
NB = N // NCORES  # batches per core = 16


def _ap(ap, dims):
    """Rebuild an AP with explicit [stride, count] free dims."""
    return bass.AP(tensor=ap.tensor, offset=ap.offset,
                   ap=[ap.ap[0]] + [list(d) for d in dims])


def build_graph(cap: int, T: float) -> bass.Bass:
    """One-core SPMD graph; cap = padded pairs per batch; T = temperature."""
    C = NB * cap          # padded pairs per core
    NT = C // 128         # 128-pair tiles
    BPT = 128 // cap      # batches per tile
    assert NT * 128 == C and BPT * cap == 128

    nc = bacc.Bacc(None, target_bir_lowering=False, debug=False)

    xt = nc.declare_dram_parameter("xt", [D, NB * K], FP8, isOutput=False)
    ancT = nc.declare_dram_parameter("ancT", [D, C], FP8, isOutput=False)
    posT = nc.declare_dram_parameter("posT", [D, C], FP8, isOutput=False)
    phrT = nc.declare_dram_parameter("phrT", [D, M], FP8, isOutput=False)
    anc = nc.declare_dram_parameter("anc", [128, NT * D], FP8, isOutput=False)
    neg3 = nc.declare_dram_parameter("neg3", [128, NT * 3 * D], FP8, isOutput=False)
    outf = nc.declare_dram_parameter("outf", [128, 2 * NT], F32, isOutput=True)
    outs = nc.declare_dram_parameter("outs", [128, 3 * NT], BF16, isOutput=True)

    with TileContext(nc) as tc:
        with (
            tc.tile_pool(name="big", bufs=1) as big,
            tc.tile_pool(name="work", bufs=2) as work,
            tc.tile_pool(name="small", bufs=8) as small,
            tc.tile_pool(name="prow", bufs=3, space="PSUM") as prow,
            tc.tile_pool(name="pce", bufs=2, space="PSUM") as pce,
        ):
            # ---- persistent tiles ----
            xt_sb = big.tile([128, NB * K], FP8, tag="xt")
            ancT_sb = big.tile([128, C], FP8, tag="ancT")
            posT_sb = big.tile([128, C], FP8, tag="posT")
            phrT_sb = big.tile([128, M], FP8, tag="phrT")
            anc_sb = big.tile([128, NT * D], FP8, tag="anc")
            neg3_sb = big.tile([128, NT * 3 * D], FP8, tag="neg3")
            prod = big.tile([128, NT * 3 * D], BF16, tag="prod")
            t8_all = big.tile([128, NT * 8], F32, tag="t8")
            sdots = big.tile([128, NT * 3], BF16, tag="sdots")
            outf_sb = big.tile([128, 2 * NT], F32, tag="outf")

            # ---- DMA issues, spread across sequencers ----
            # sync: CE stationaries first (CE matmuls warm the PE), then the
            # sim stationaries + first xt chunks
            KC = NB * K // 4
            nc.sync.dma_start(out=posT_sb, in_=posT[:, :])
            nc.sync.dma_start(out=phrT_sb, in_=phrT[:, :])
            nc.sync.dma_start(out=ancT_sb, in_=ancT[:, :])
            nc.sync.dma_start(out=xt_sb[:, 0:KC], in_=xt[:, 0:KC])
            nc.sync.dma_start(out=xt_sb[:, KC:2 * KC], in_=xt[:, KC:2 * KC])
            # scalar: xt tail (only SP/Activation/Pool can issue DMAs)
            nc.scalar.dma_start(out=xt_sb[:, 2 * KC:3 * KC],
                                in_=xt[:, 2 * KC:3 * KC])
            nc.scalar.dma_start(out=xt_sb[:, 3 * KC:4 * KC],
                                in_=xt[:, 3 * KC:4 * KC])
            # pool: triplet dot operands (feed the GpSimd products)
            nc.gpsimd.dma_start(out=anc_sb, in_=anc[:, :])
            nc.gpsimd.dma_start(out=neg3_sb, in_=neg3[:, :])

            # ---- GpSimd: anchor * [pos|rng0|rng1], one op per tile ----
            for t in range(NT):
                anc_b3 = _ap(anc_sb[:, t * D:(t + 1) * D], [[0, 3], [1, D]])
                nc.gpsimd.tensor_mul(
                    prod[:, t * 3 * D:(t + 1) * 3 * D].rearrange(
                        "p (r d) -> p r d", r=3),
                    anc_b3,
                    neg3_sb[:, t * 3 * D:(t + 1) * 3 * D].rearrange(
                        "p (r d) -> p r d", r=3))

            def ce_mm(t):
                lg = pce.tile([128, 512], F32, tag="lg")
                nc.tensor.matmul(lg, posT_sb[:, t * 128:(t + 1) * 128],
                                 phrT_sb, start=True, stop=True)
                je = work.tile([128, 512], BF16, tag="je")
                nc.scalar.activation(je, lg, AF.Exp, scale=float(T),
                                     accum_out=outf_sb[:, NT + t:NT + t + 1])

            def sim_mm(t):
                rp = prow.tile([128, 1024], F32, tag="rp")
                for h in range(BPT):
                    b = BPT * t + h
                    acols = ancT_sb[:, b * cap:(b + 1) * cap]
                    for g in range(K // 512):
                        nc.tensor.matmul(
                            rp[cap * h:cap * (h + 1), g * 512:(g + 1) * 512],
                            acols,
                            xt_sb[:, b * K + g * 512:b * K + (g + 1) * 512],
                            start=True, stop=True)
                nc.vector.max(t8_all[:, t * 8:(t + 1) * 8], rp)
                # interleave one triplet-dot reduce chunk behind each Max8
                with nc.allow_low_precision("bf16 triplet dot sums"):
                    nc.vector.tensor_reduce(
                        sdots[:, t * 3:(t + 1) * 3],
                        prod[:, t * 3 * D:(t + 1) * 3 * D].rearrange(
                            "p (r d) -> p r d", r=3),
                        AX.X, ALU.add)

            # PE order: a few CE matmuls first (tiny DMA deps; they warm the
            # p-state), then interleave sims as xt chunks land.
            ce_mm(0); ce_mm(1); ce_mm(2)
            nxt = 3
            for t in range(NT):
                sim_mm(t)
                if nxt < NT:
                    ce_mm(nxt)
                    nxt += 1

            # ---- finale: triplet hinge terms (CE finale runs on host) ----
            spos4 = _ap(sdots[:, :], [[3, NT], [0, 4]])
            t84 = _ap(t8_all[:, :], [[8, NT], [1, 4]])
            u4t = small.tile([128, NT * 4], F32, tag="u4t")
            nc.vector.scalar_tensor_tensor(
                u4t[:, :].rearrange("p (t e) -> p t e", e=4), t84, 1.0, spos4,
                op0=ALU.add, op1=ALU.subtract)
            nc.vector.tensor_scalar_max(u4t, u4t, 0.0)
            s4 = small.tile([128, NT], F32, tag="s4")
            nc.vector.tensor_reduce(
                s4, u4t[:, :].rearrange("p (t e) -> p t e", e=4), AX.X, ALU.add)
            w = small.tile([128, NT], F32, tag="w")
            u4th = _ap(u4t[:, 3:4], [[4, NT]])
            nc.vector.tensor_scalar_max(w, u4th, 1.0)

            srnd = _ap(sdots[:, 1:2], [[3, NT], [1, 2]])
            spos2 = _ap(sdots[:, :], [[3, NT], [0, 2]])
            ur = small.tile([128, NT * 2], F32, tag="ur")
            nc.vector.scalar_tensor_tensor(
                ur[:, :].rearrange("p (t e) -> p t e", e=2), srnd, 1.0, spos2,
                op0=ALU.add, op1=ALU.subtract)
            nc.vector.tensor_scalar_max(ur, ur, 0.0)
            r2 = small.tile([128, NT], F32, tag="r2")
            nc.vector.tensor_reduce(
                r2, ur[:, :].rearrange("p (t e) -> p t e", e=2), AX.X, ALU.add)

            hard = small.tile([128, NT], F32, tag="hard")
            nc.vector.tensor_sub(hard, s4, w)
            nc.vector.tensor_add(outf_sb[:, 0:NT], hard, r2)

            nc.sync.dma_start(out=outf[:, :], in_=outf_sb)
            nc.gpsimd.dma_start(out=outs[:, :], in_=sdots)

    if not nc.is_finalized():
        nc.finalize()
    return nc


_CACHE = {}
_BF16 = mybir.dt.np(BF16)
_FP8 = mybir.dt.np(FP8)


def _l2n(x):
    return x / np.maximum(np.linalg.norm(x, axis=-1, keepdims=True), 1e-12)


def _tiled(a, ntiles, width):
    """[ntiles*128, width] -> [128, ntiles*width] device tile layout."""
    return np.ascontiguousarray(
        a.reshape(ntiles, 128, width).transpose(1, 0, 2).reshape(128, ntiles * width))


def _prep_core(c, cap, pe, ie, bi, mi, ki, rn):
    """pe/ie are pre-normalized f32."""
    C = NB * cap
    NT = C // 128
    lo = NB * c
    sel = np.where((bi >= lo) & (bi < lo + NB))[0]
    # pad with unit vectors (already normalized)
    ancb = np.zeros((C, D), np.float32); ancb[:, 0] = 1.0
    posb = np.zeros((C, D), np.float32); posb[:, 0] = 1.0
    rngb = np.zeros((C, 2, D), np.float32); rngb[:, :, 0] = 1.0
    valid = np.zeros(C, np.float32)
    for n in range(NB):
        pb = sel[bi[sel] == lo + n]
        assert len(pb) <= cap
        s = n * cap
        ancb[s:s + len(pb)] = pe[mi[pb]]
        posb[s:s + len(pb)] = ie[bi[pb], ki[pb]]
        rngb[s:s + len(pb), 0] = ie[bi[pb], rn[pb, 0]]
        rngb[s:s + len(pb), 1] = ie[bi[pb], rn[pb, 1]]
        valid[s:s + len(pb)] = 1.0
    xt_c = np.ascontiguousarray(
        ie[lo:lo + NB].reshape(NB * K, D).T).astype(_FP8)
    neg = np.concatenate([posb[:, None, :], rngb], axis=1)  # [C, 3, D]
    vt = np.ascontiguousarray(valid.reshape(NT, 128).T)     # [128, NT]
    return dict(
        xt=xt_c,
        ancT=np.ascontiguousarray(ancb.T).astype(_FP8),
        posT=np.ascontiguousarray(posb.T).astype(_FP8),
        phrT=np.ascontiguousarray(pe.T).astype(_FP8),
        anc=_tiled(ancb.astype(_FP8), NT, D),
        neg3=_tiled(neg.reshape(C, 3 * D).astype(_FP8), NT, 3 * D),
    ), vt


def make_in_maps(inputs, cap=None):
    pe = _l2n(np.asarray(inputs["phrase_embeddings"], np.float32))
    ie = _l2n(np.asarray(inputs["input_embeddings"], np.float32))
    bi = np.asarray(inputs["batch_idxs"])
    mi = np.asarray(inputs["phrase_emb_idxs"])
    ki = np.asarray(inputs["input_emb_idxs"])
    rn = np.asarray(inputs["rand_neg_idx"])
    T = float(np.asarray(inputs["temperature"]))
    if cap is None:
        maxc = int(np.bincount(bi, minlength=N).max())
        cap = max(64, ((maxc + 63) // 64) * 64)
    maps, vts = [], []
    for c in range(NCORES):
        m, vt = _prep_core(c, cap, pe, ie, bi, mi, ki, rn)
        maps.append(m)
        vts.append(vt)
    return maps, vts, cap, T


def kernel(**inputs):
    in_maps, vts, cap, T = make_in_maps(inputs)
    key = (cap, T)
    if key not in _CACHE:
        _CACHE[key] = build_graph(cap, T)
    nc = _CACHE[key]
    res = run_bass_kernel_spmd(nc, in_maps, core_ids=list(range(NCORES)))
    NT = NB * cap // 128
    trip_sum = 0.0
    ce_sum = 0.0
    for c, r in enumerate(res.results):
        of = np.asarray(r["outf"], np.float32)        # [128, 2NT]
        sd = np.asarray(r["outs"]).astype(np.float32)  # [128, 3NT]
        vt = vts[c]                                    # [128, NT]
        trip_sum += float((of[:, :NT] * vt).sum())
        spos = sd[:, 0::3]                             # [128, NT]
        ce_t = np.log(of[:, NT:2 * NT]) - T * spos
        ce_sum += float((ce_t * vt).sum())
    trip = trip_sum / (P * 5)
    ce = ce_sum / P
    return np.float32(trip), np.float32(ce)


# revision 8
# speedup vs baseline: 1.3285x; 1.3285x over previous
"""Trainium2 Bass kernel for nn_AlignmentLoss (triplet + CE over phrase/input embeddings).

Sharding: batch dimension N=128 split 16 batches/core across 8 cores.  Each core
owns the positive pairs whose batch_idxs falls in its range (host buckets pairs,
padded to a fixed per-batch capacity cap=64; 2 batches share a 128-partition tile).

v7 design:
 - Host L2-normalizes phrase and input embeddings in f32 (exactly the
   reference's F.normalize preprocessing), so the device never computes
   norms.
 - Device computes the two O(P*K*D)/O(P*M*D) tensor contractions and the
   per-pair reductions over them: sim rows (fp8 matmul) -> DVE Max8 top-8
   mining straight from PSUM, and CE logits (fp8 matmul) -> ACT
   Exp(scale=T) with accum_out sum over the M phrases.
 - Per-pair outputs (top-8 sims, sum-exp) DMA back; the host applies the
   O(P) hinge/log finale and the valid-pair masking/means.
 - xt streams in 8 chunks spread over the three DMA queues (sync q1,
   scalar q10, pool q0; each queue is ~160GB/s); the small CE/sim
   stationaries ride in front of scalar's queue so the PE starts early.
"""

import sys

for _p in ("/opt/trn_rl_repo", "/root/.axon_site/_ro/trn_rl_repo"):
    if _p not in sys.path:
        sys.path.append(_p)

import numpy as np

import concourse.bass as bass
import concourse.bacc as bacc
import concourse.mybir as mybir
from concourse.tile import TileContext
from concourse.bass_utils import run_bass_kernel_spmd

F32 = mybir.dt.float32
BF16 = mybir.dt.bfloat16
FP8 = mybir.dt.float8e4
AF = mybir.ActivationFunctionType
ALU = mybir.AluOpType
AX = mybir.AxisListType

N, K, M, D, P = 128, 1024, 512, 128, 4096
NCORES = 8
NB = N // NCORES  # batches per core = 16


def build_graph(cap: int, T: float) -> bass.Bass:
    """One-core SPMD graph; cap = padded pairs per batch; T = temperature."""
    C = NB * cap          # padded pairs per core
    NT = C // 128         # 128-pair tiles
    BPT = 128 // cap      # batches per tile
    assert NT * 128 == C and BPT * cap == 128

    nc = bacc.Bacc(None, target_bir_lowering=False, debug=False)

    xt = nc.declare_dram_parameter("xt", [D, NB * K], FP8, isOutput=False)
    ancT = nc.declare_dram_parameter("ancT", [D, C], FP8, isOutput=False)
    posT = nc.declare_dram_parameter("posT", [D, C], FP8, isOutput=False)
    phrT = nc.declare_dram_parameter("phrT", [D, M], FP8, isOutput=False)
    out = nc.declare_dram_parameter("out", [128, 9 * NT], F32, isOutput=True)

    KB = BPT * K          # xt columns per tile-chunk (2048)

    with TileContext(nc) as tc:
        with (
            tc.tile_pool(name="big", bufs=1) as big,
            tc.tile_pool(name="work", bufs=2) as work,
            tc.tile_pool(name="prow", bufs=3, space="PSUM") as prow,
            tc.tile_pool(name="pce", bufs=2, space="PSUM") as pce,
        ):
            # ---- persistent tiles ----
            xt_sb = big.tile([128, NB * K], FP8, tag="xt")
            ancT_sb = big.tile([128, C], FP8, tag="ancT")
            posT_sb = big.tile([128, C], FP8, tag="posT")
            phrT_sb = big.tile([128, M], FP8, tag="phrT")
            out_sb = big.tile([128, 9 * NT], F32, tag="out")

            # ---- DMA issues over the three queues ----
            def xt_chunk(eng, t):
                eng.dma_start(out=xt_sb[:, t * KB:(t + 1) * KB],
                              in_=xt[:, t * KB:(t + 1) * KB])

            # scalar q10: the small stationaries alone, then late xt chunks
            nc.scalar.dma_start(out=posT_sb, in_=posT[:, :])
            nc.scalar.dma_start(out=phrT_sb, in_=phrT[:, :])
            nc.scalar.dma_start(out=ancT_sb, in_=ancT[:, :])
            # sync q1 / pool q0: xt in tile-consumption order
            xt_chunk(nc.sync, 0)
            xt_chunk(nc.gpsimd, 1)
            xt_chunk(nc.sync, 2)
            xt_chunk(nc.gpsimd, 3)
            xt_chunk(nc.scalar, 4)
            xt_chunk(nc.sync, 5)
            xt_chunk(nc.gpsimd, 6)
            xt_chunk(nc.scalar, 7)

            def ce_mm(t):
                lg = pce.tile([128, 512], F32, tag="lg")
                nc.tensor.matmul(lg, posT_sb[:, t * 128:(t + 1) * 128],
                                 phrT_sb, start=True, stop=True)
                je = work.tile([128, 512], BF16, tag="je")
                nc.scalar.activation(je, lg, AF.Exp, scale=float(T),
                                     accum_out=out_sb[:, 8 * NT + t:8 * NT + t + 1])

            def sim_mm(t):
                rp = prow.tile([128, 1024], F32, tag="rp")
                for h in range(BPT):
                    b = BPT * t + h
                    acols = ancT_sb[:, b * cap:(b + 1) * cap]
                    for g in range(K // 512):
                        nc.tensor.matmul(
                            rp[cap * h:cap * (h + 1), g * 512:(g + 1) * 512],
                            acols,
                            xt_sb[:, b * K + g * 512:b * K + (g + 1) * 512],
                            start=True, stop=True)
                nc.vector.max(out_sb[:, t * 8:(t + 1) * 8], rp)

            # PE order: a few CE matmuls first (tiny DMA deps; they warm the
            # p-state), then interleave sims as xt chunks land.
            ce_mm(0); ce_mm(1); ce_mm(2)
            nxt = 3
            for t in range(NT):
                sim_mm(t)
                if nxt < NT:
                    ce_mm(nxt)
                    nxt += 1

            nc.sync.dma_start(out=out[:, :], in_=out_sb)

    if not nc.is_finalized():
        nc.finalize()
    return nc


_CACHE = {}
_FP8 = mybir.dt.np(FP8)


def _l2n(x):
    return x / np.maximum(np.linalg.norm(x, axis=-1, keepdims=True), 1e-12)


def _prep_core(c, cap, pe, ie, bi, mi, ki, rn):
    """pe/ie are pre-normalized f32.  Returns (device map, host-side stats)."""
    C = NB * cap
    NT = C // 128
    lo = NB * c
    sel = np.where((bi >= lo) & (bi < lo + NB))[0]
    # pad with unit vectors (already normalized)
    ancb = np.zeros((C, D), np.float32); ancb[:, 0] = 1.0
    posb = np.zeros((C, D), np.float32); posb[:, 0] = 1.0
    rngb = np.zeros((C, 2, D), np.float32); rngb[:, :, 0] = 1.0
    valid = np.zeros(C, np.float32)
    for n in range(NB):
        pb = sel[bi[sel] == lo + n]
        assert len(pb) <= cap
        s = n * cap
        ancb[s:s + len(pb)] = pe[mi[pb]]
        posb[s:s + len(pb)] = ie[bi[pb], ki[pb]]
        rngb[s:s + len(pb), 0] = ie[bi[pb], rn[pb, 0]]
        rngb[s:s + len(pb), 1] = ie[bi[pb], rn[pb, 1]]
        valid[s:s + len(pb)] = 1.0
    xt_c = np.ascontiguousarray(
        ie[lo:lo + NB].reshape(NB * K, D).T).astype(_FP8)
    dev = dict(
        xt=xt_c,
        ancT=np.ascontiguousarray(ancb.T).astype(_FP8),
        posT=np.ascontiguousarray(posb.T).astype(_FP8),
        phrT=np.ascontiguousarray(pe.T).astype(_FP8),
    )
    # host-side per-pair stats in [128, NT] tile layout (tile t, partition p
    # <-> pair t*128+p), matching the device's Max8 output layout
    spos = np.einsum('cd,cd->c', ancb, posb).reshape(NT, 128).T
    srnd = np.einsum('cd,crd->cr', ancb, rngb).reshape(NT, 128, 2).transpose(1, 0, 2)
    vt = valid.reshape(NT, 128).T
    return dev, (spos, srnd, vt)


def make_in_maps(inputs, cap=None):
    pe = _l2n(np.asarray(inputs["phrase_embeddings"], np.float32))
    ie = _l2n(np.asarray(inputs["input_embeddings"], np.float32))
    bi = np.asarray(inputs["batch_idxs"])
    mi = np.asarray(inputs["phrase_emb_idxs"])
    ki = np.asarray(inputs["input_emb_idxs"])
    rn = np.asarray(inputs["rand_neg_idx"])
    T = float(np.asarray(inputs["temperature"]))
    if cap is None:
        maxc = int(np.bincount(bi, minlength=N).max())
        cap = max(64, ((maxc + 63) // 64) * 64)
    maps, stats = [], []
    for c in range(NCORES):
        m, st = _prep_core(c, cap, pe, ie, bi, mi, ki, rn)
        maps.append(m)
        stats.append(st)
    return maps, stats, cap, T


def kernel(**inputs):
    in_maps, stats, cap, T = make_in_maps(inputs)
    key = (cap, T)
    if key not in _CACHE:
        _CACHE[key] = build_graph(cap, T)
    nc = _CACHE[key]
    res = run_bass_kernel_spmd(nc, in_maps, core_ids=list(range(NCORES)))
    NT = NB * cap // 128
    trip_sum = 0.0
    ce_sum = 0.0
    for c, r in enumerate(res.results):
        of = np.asarray(r["out"], np.float32)            # [128, 9NT]
        t8 = of[:, :8 * NT].reshape(128, NT, 8)
        sumexp = of[:, 8 * NT:9 * NT]                    # [128, NT]
        spos, srnd, vt = stats[c]
        u = np.maximum(t8[:, :, :4] + 1.0 - spos[:, :, None], 0.0)
        s4 = u.sum(-1)
        w = np.maximum(u[:, :, 3], 1.0)
        r2 = np.maximum(srnd + 1.0 - spos[:, :, None], 0.0).sum(-1)
        trip_sum += float(((s4 - w + r2) * vt).sum())
        ce_t = np.log(sumexp) - T * spos
        ce_sum += float((ce_t * vt).sum())
    trip = trip_sum / (P * 5)
    ce = ce_sum / P
    return np.float32(trip), np.float32(ce)
